# revision 30
# baseline (speedup 1.0000x reference)
"""Trainium2 Bass kernel for nn_CausalSelfAttention_59253368815644.

Sharding: 8 cores = 2 (batch) x 4 (head groups of 4 heads). Per core:
qkv projection (bf16 matmuls, FWL), rms-norm via DVE squares + PE
partition-sums + ACT ln/exp (single activation table set), rotary in bf16
on DVE with direct writes into persistent Q/K tiles (KEY_OFFSET via
split-destination writes -- no scatter DMAs), doc-masked causal attention
(one-hot augmented contraction rows; causal via affine_select), softmax
without max-subtraction, gated value embedding, attention output gate
(sigmoid via exp + fast reciprocal), partial output projection. Host sums
4 partials per batch element.
"""
import sys

sys.path.insert(0, "/opt/trn_rl_repo")

from contextlib import ExitStack

import ml_dtypes
import numpy as np

import concourse.bass as bass
import concourse.tile as tile
from concourse import bacc, mybir
from concourse._compat import with_exitstack
from concourse.bass_utils import run_bass_kernel_spmd

F32 = mybir.dt.float32
F32R = mybir.dt.float32r
BF16 = mybir.dt.bfloat16
AF = mybir.ActivationFunctionType
BF = ml_dtypes.bfloat16

B, T, D, H, HD = 2, 2048, 1024, 16, 64
EPS = 1.1920929e-07
VE_GATE_SCALE = 2.0
NHEADS = 4          # heads per core
HGROUPS = 4
NCHUNK = D // 128   # 8 contraction chunks
TTILE = 512
NTT = T // TTILE
BIG = 30.0          # mask exponent after exp-scale
NDOC = 8
AUG = NDOC + 1
QR = 64 + AUG       # 73 partitions for Q^/K^


def build_spans(segs):
    """Greedy partition of [0,T) into q-spans (len 256..512 where possible),
    preferring doc-boundary ends. Returns [(a, b, kts)]."""
    bounds = [e for (_, e) in segs]
    spans = []
    a = 0
    while a < T:
        cands = [e for e in bounds if a < e <= a + 512]
        end = None
        if cands:
            mx = max(cands)
            if mx - a >= 256 or mx == T:
                end = mx
        if end is None:
            end = min(a + 512, T)
        if end % 2 != 0 and end < T:
            end += 1
        ks = max((s for (s, _) in segs if s <= a), default=0)
        spans.append((a, end, ks))
        a = end
    def docend(pos):
        for (s_, e_) in segs:
            if s_ <= pos < e_:
                return e_
        return T

    out = []
    for (a, b, ks) in spans:
        ka0 = (ks // 128) * 128
        kts = []
        ka = ka0
        while ka < b:
            kn = min(128, b - ka)
            # q-columns beyond the last key row's document are fully masked
            qhi = min(b - a, docend(ka + kn - 1) - a)
            w0 = max(0, ka - a)
            if qhi > w0:
                kts.append((ka, kn, (ka + kn) > a, qhi))
            ka += 128
        out.append((a, b, kts))
    return out


@with_exitstack
def build_kernel(ctx: ExitStack, tc: tile.TileContext, dr, spans, alpha):
    nc = tc.nc

    const = ctx.enter_context(tc.tile_pool(name="const", bufs=1))
    persist = ctx.enter_context(tc.tile_pool(name="persist", bufs=1))

    # ---- constants / inputs staged in SBUF ----
    # queue order matters: wqk + x tile 0 first so the first qk matmuls can
    # start as early as possible; veb is not needed until the first vps gate
    wqk = const.tile([128, NCHUNK, 512], BF16)
    nc.scalar.dma_start(wqk[:],
                        dr["wqk"][:].rearrange("p (c e) -> p c e", e=512))
    wv = const.tile([128, NCHUNK, 260], BF16)
    nc.gpsimd.dma_start(wv[:],
                        dr["wv"][:].rearrange("p (c e) -> p c e", e=260))
    wga36 = const.tile([128, NCHUNK, 36], BF16)
    nc.gpsimd.dma_start(wga36[:],
                        dr["wga36"][:].rearrange("p (c e) -> p c e", e=36))
    xall = const.tile([128, NTT, NCHUNK, TTILE], BF16)
    xr = dr["xT"][:].rearrange("p (u c t) -> p u c t", c=NCHUNK, t=TTILE)
    for u in range(NTT):
        (nc.sync, nc.scalar)[u % 2].dma_start(xall[:, u, :, :], xr[:, u, :, :])
    vesb = const.tile([128, T // 128, 256], BF16)
    nc.scalar.dma_start(
        vesb[:], dr["veb"][:].rearrange("p (s e) -> p s e", e=256))
    cdup = const.tile([128, T], BF16)
    nc.gpsimd.dma_start(cdup[:], dr["cdup"][:])
    s2dup = const.tile([128, T], BF16)
    nc.gpsimd.dma_start(s2dup[:], dr["s2dup"][:])
    onesEt = const.tile([128, NHEADS, 8], BF16)
    nc.scalar.dma_start(onesEt[:],
                        dr["onesEt"][:].rearrange("p (b e) -> p b e", e=8))
    e8sel = const.tile([8, NHEADS, 128], BF16)
    nc.scalar.dma_start(e8sel[:],
                        dr["e8sel"][:].rearrange("p (b e) -> p b e", e=128))
    e4a = const.tile([4, 128], BF16)
    nc.scalar.dma_start(e4a[:], dr["e4"][0, :, :])
    e4b = const.tile([4, 128], BF16)
    nc.scalar.dma_start(e4b[:], dr["e4"][1, :, :])
    epsb = const.tile([8, 1], F32)
    nc.vector.memset(epsb[:], EPS)
    negio = const.tile([128, 512], F32)
    nc.sync.dma_start(negio[:], dr["negio"][:])
    negp = const.tile([128, 1], F32)
    nc.sync.dma_start(negp[:], dr["negp"][:])
    wo = const.tile([128, 2, 1024], BF16)
    nc.sync.dma_start(wo[:],
                      dr["wo"][:].rearrange("p (c e) -> p c e", e=1024))

    # ---- persistent activations ----
    Qh = persist.tile([QR, NHEADS, T], BF16)
    Kh = persist.tile([QR, NHEADS, T], BF16)
    qaug = dr["qaug"]
    kaug = dr["kaug"]
    nc.gpsimd.dma_start(
        Qh[64:QR, :, :],
        bass.AP(tensor=qaug.tensor, offset=qaug.offset,
                ap=[[T, AUG], [0, NHEADS], [1, T]]))
    nc.gpsimd.dma_start(
        Kh[64:QR, :, :],
        bass.AP(tensor=kaug.tensor, offset=kaug.offset,
                ap=[[T, AUG], [0, NHEADS], [1, T]]))
    Vh = persist.tile([128, T // 128, NHEADS, 65], BF16)
    nc.vector.memset(Vh[:, :, :, 64:65], 1.0)
    agrow1 = persist.tile([NHEADS, T], F32)   # 1 + exp(-attn_gate_logit)

    # =========== Phase 1 ===========
    with tc.tile_pool(name="p1qk", bufs=1, space="PSUM") as qkps_pool, \
         tc.tile_pool(name="p1ss", bufs=1, space="PSUM") as ssps_pool, \
         tc.tile_pool(name="p1v", bufs=2, space="PSUM") as vps_pool, \
         tc.tile_pool(name="p1rb", bufs=1, space="PSUM") as rbps_pool, \
         tc.tile_pool(name="p1sb", bufs=2) as sb_pool, \
         tc.tile_pool(name="p1n", bufs=2) as n1_pool:
        for tt in range(NTT):
            t0 = tt * TTILE

            # qkv projection for 4 heads (q|k interleaved per head)
            qk = qkps_pool.tile([128, NHEADS, TTILE], F32, tag="qk")
            for blk in range(NHEADS):
                for c in range(NCHUNK):
                    nc.tensor.matmul(
                        qk[:, blk, :], wqk[:, c, blk * 128:(blk + 1) * 128],
                        xall[:, tt, c, :],
                        start=(c == 0), stop=(c == NCHUNK - 1))
            qkb = sb_pool.tile([128, NHEADS, TTILE], BF16, tag="qkb")
            nc.scalar.activation(out=qkb[:], in_=qk[:], func=AF.Copy,
                                 scale=1.0)
            sq = sb_pool.tile([128, NHEADS, TTILE], BF16, tag="sq")
            nc.vector.tensor_mul(sq[:], qkb[:], qkb[:])

            # attn-gate logits (rows 32:36) + per-head sum-squares (rows 0:8)
            ss8z = ssps_pool.tile([36, TTILE], F32, tag="ss")
            for c in range(NCHUNK):
                nc.tensor.matmul(ss8z[:], wga36[:, c, :], xall[:, tt, c, :],
                                 start=(c == 0), stop=False)
            for blk in range(NHEADS):
                nc.tensor.matmul(ss8z[0:8, :], onesEt[:, blk, :],
                                 sq[:, blk, :],
                                 start=False, stop=(blk == NHEADS - 1))
            lnss = n1_pool.tile([8, TTILE], F32, tag="lnss")
            nc.scalar.activation(out=lnss[:], in_=ss8z[0:8, :], func=AF.Ln,
                                 scale=1.0 / HD, bias=epsb[:])
            rstd8 = n1_pool.tile([8, TTILE], BF16, tag="rstd8")
            nc.scalar.activation(out=rstd8[:], in_=lnss[:], func=AF.Exp,
                                 scale=-0.5)
            age = n1_pool.tile([NHEADS, TTILE], F32, tag="age")
            nc.scalar.activation(out=age[:], in_=ss8z[32:36, :], func=AF.Exp,
                                 scale=-1.0)
            nc.vector.tensor_scalar_add(agrow1[:, t0:t0 + TTILE], age[:], 1.0)

            for blk in range(NHEADS):
                rbps = rbps_pool.tile([128, TTILE], F32, tag="rb")
                nc.tensor.matmul(rbps[:], e8sel[:, blk, :], rstd8[:],
                                 start=True, stop=True)
                rstdb = n1_pool.tile([128, TTILE], BF16, tag="rstdb")
                nc.scalar.activation(out=rstdb[:], in_=rbps[:], func=AF.Copy,
                                     scale=1.0)
                A = sb_pool.tile([128, TTILE], BF16, tag="A")
                nc.vector.tensor_mul(A[:], qkb[:, blk, :],
                                     cdup[:, t0:t0 + TTILE])
                Bt = sb_pool.tile([128, TTILE], BF16, tag="B")
                nc.vector.tensor_mul(Bt[:], qkb[:, blk, :],
                                     s2dup[:, t0:t0 + TTILE])
                Bs = sb_pool.tile([128, TTILE], BF16, tag="Bs")
                nc.vector.stream_shuffle(Bs[:], Bt[:],
                                         mask=[g ^ 16 for g in range(32)])
                rotr = sb_pool.tile([128, TTILE], BF16, tag="rotr")
                nc.vector.tensor_add(rotr[:], A[:], Bs[:])
                h = blk
                nc.vector.tensor_mul(Qh[0:64, h, t0:t0 + TTILE],
                                     rotr[0:64, :], rstdb[0:64, :])
                nc.vector.tensor_mul(Kh[0:32, h, t0:t0 + TTILE],
                                     rotr[64:96, :], rstdb[64:96, :])
                w = TTILE if t0 + TTILE < T else TTILE - 1
                nc.vector.tensor_mul(Kh[32:64, h, t0 + 1:t0 + 1 + w],
                                     rotr[96:128, 0:w], rstdb[96:128, 0:w])
                if t0 == 0:
                    nc.vector.tensor_mul(Kh[32:64, h, 0:1],
                                         rotr[96:128, 0:1], rstdb[96:128, 0:1])

            # value projection + gated ve
            for sub in range(TTILE // 128):
                stg = (t0 + sub * 128) // 128
                vps = vps_pool.tile([128, 260], F32, tag="v")
                for c in range(NCHUNK):
                    nc.tensor.matmul(
                        vps[:], xall[:, tt, c, sub * 128:(sub + 1) * 128],
                        wv[:, c, :],
                        start=(c == 0), stop=(c == NCHUNK - 1))
                ge = n1_pool.tile([128, NHEADS], F32, tag="ge")
                nc.scalar.activation(out=ge[:], in_=vps[:, 256:260],
                                     func=AF.Exp, scale=-1.0)
                nc.vector.tensor_scalar_add(ge[:], ge[:], 1.0)
                gf = n1_pool.tile([128, NHEADS], F32, tag="gf")
                nc.vector.reciprocal_approx_fast(out=gf[:], in_=ge[:])
                gb16 = n1_pool.tile([128, NHEADS], BF16, tag="gb16")
                nc.vector.tensor_copy(gb16[:], gf[:])
                gap = gb16[:]
                gb = bass.AP(tensor=gap.tensor, offset=gap.offset,
                             ap=[list(gap.ap[0]), [1, NHEADS], [0, HD]])
                tmp = n1_pool.tile([128, NHEADS, HD], BF16, tag="vtmp")
                nc.gpsimd.tensor_mul(
                    tmp[:],
                    vesb[:, stg, :].rearrange("p (h d) -> p h d", h=NHEADS),
                    gb)
                nc.vector.tensor_add(
                    Vh[:, stg, :, 0:64],
                    vps[:, 0:256].rearrange("p (h d) -> p h d", h=NHEADS),
                    tmp[:])

    # =========== Phase 2 (attention + interleaved o-proj) ===========
    ypool = ctx.enter_context(tc.tile_pool(name="ylate", bufs=1))
    y01 = ypool.tile([128, T], BF16)
    y23 = ypool.tile([128, T], BF16)

    def oproj(ti, ops_pool, osb_pool):
        tt0 = ti * 128
        for eh in range(2):
            ops = ops_pool.tile([128, 512], F32, tag="o")
            nc.tensor.matmul(ops[:], y01[:, tt0:tt0 + 128],
                             wo[:, 0, eh * 512:(eh + 1) * 512],
                             start=True, stop=False)
            nc.tensor.matmul(ops[:], y23[:, tt0:tt0 + 128],
                             wo[:, 1, eh * 512:(eh + 1) * 512],
                             start=False, stop=True)
            osb = osb_pool.tile([128, 512], F32, tag="osb")
            nc.vector.tensor_copy(osb[:], ops[:])
            nc.sync.dma_start(
                dr["out"][tt0:tt0 + 128, eh * 512:(eh + 1) * 512], osb[:])

    with tc.tile_pool(name="p2s", bufs=3, space="PSUM") as sps_pool, \
         tc.tile_pool(name="p2y", bufs=1, space="PSUM") as yps_pool, \
         tc.tile_pool(name="p3ps", bufs=1, space="PSUM") as ops_pool, \
         tc.tile_pool(name="p3sb", bufs=3) as osb_pool, \
         tc.tile_pool(name="p2p", bufs=8) as pt_pool, \
         tc.tile_pool(name="p2sc", bufs=2) as sc_pool:
        bps_pool = ops_pool
        pending = []
        onext = 0
        for (a, b_, kts) in spans:
            N = b_ - a
            ypss = []
            for h in range(NHEADS):
                # paced o-proj filler: one finished 128-token tile per head
                # chain keeps the PE fed through exp/affine stalls
                if pending:
                    oproj(pending.pop(0), ops_pool, osb_pool)
                yps = yps_pool.tile([65, 512], F32, tag=f"y{h}")
                ypss.append(yps)
                for ki, (ka, kn, causal, qhi) in enumerate(kts):
                    w0 = max(0, ka - a)
                    sps = sps_pool.tile([128, 512], F32, tag="s")
                    nc.tensor.matmul(sps[0:kn, w0:qhi],
                                     Kh[:, h, ka:ka + kn],
                                     Qh[:, h, a + w0:a + qhi],
                                     start=True, stop=True)
                    pt = pt_pool.tile([128, 512], BF16, tag="p")
                    nc.scalar.activation(out=pt[0:kn, w0:qhi],
                                         in_=sps[0:kn, w0:qhi],
                                         func=AF.Exp, scale=alpha)
                    if causal:
                        bw = min(qhi, ka + kn - a) - w0
                        if bw > 0:
                            nc.gpsimd.affine_select(
                                out=pt[0:kn, w0:w0 + bw],
                                in_=pt[0:kn, w0:w0 + bw],
                                compare_op=mybir.AluOpType.is_ge,
                                fill=0.0, base=a + w0 - ka,
                                pattern=[[1, bw]], channel_multiplier=-1)
                    nc.tensor.matmul(yps[:, w0:qhi],
                                     Vh[0:kn, ka // 128, h, :],
                                     pt[0:kn, w0:qhi],
                                     start=(ki == 0), stop=(ki == len(kts) - 1))
            l4 = sc_pool.tile([NHEADS, 512], F32, tag="l4")
            for h in range(NHEADS):
                l1 = sc_pool.tile([1, 512], F32, tag=f"l1_{h}")
                nc.vector.tensor_copy(l1[:, 0:N], ypss[h][64:65, 0:N])
                nc.sync.dma_start(l4[h:h + 1, 0:N], l1[:, 0:N])
            mm = sc_pool.tile([NHEADS, 512], F32, tag="mm")
            nc.vector.tensor_mul(mm[:, 0:N], l4[:, 0:N], agrow1[:, a:b_])
            scf = sc_pool.tile([NHEADS, 512], F32, tag="scf")
            nc.vector.reciprocal_approx_fast(out=scf[:, 0:N], in_=mm[:, 0:N])
            sc4 = sc_pool.tile([NHEADS, 512], BF16, tag="sc")
            nc.vector.tensor_copy(sc4[:, 0:N], scf[:, 0:N])
            for pr, ytile in ((0, y01), (1, y23)):
                sbcps = bps_pool.tile([128, 512], F32, tag="o")
                nc.tensor.matmul(sbcps[:, 0:N], e4a[:] if pr == 0 else e4b[:],
                                 sc4[:, 0:N], start=True, stop=True)
                sbcs = sc_pool.tile([128, 512], BF16, tag="sbcs")
                nc.scalar.activation(out=sbcs[:, 0:N], in_=sbcps[:, 0:N],
                                     func=AF.Copy, scale=1.0)
                yy = sc_pool.tile([128, 512], BF16, tag="yy")
                nc.vector.tensor_copy(yy[0:64, 0:N],
                                      ypss[2 * pr][0:64, 0:N])
                nc.vector.tensor_copy(yy[64:128, 0:N],
                                      ypss[2 * pr + 1][0:64, 0:N])
                nc.vector.tensor_mul(ytile[:, a:b_], yy[:, 0:N],
                                     sbcs[:, 0:N])
            pending.extend(range(onext, b_ // 128))
            onext = b_ // 128
        for ti in pending:
            oproj(ti, ops_pool, osb_pool)


_CACHE = {}
TRACE = False       # set by test harness to capture an NTFF profile
LAST_RESULT = None  # BassKernelResults of the most recent run


def _get_program(key, spans, alpha):
    if key in _CACHE:
        return _CACHE[key]
    nc = bacc.Bacc("TRN2", target_bir_lowering=False, debug=False)
    dr = {}

    def di(name, shape, dt=F32):
        dr[name] = nc.dram_tensor(name, shape, dt, kind="ExternalInput").ap()

    di("xT", [128, NTT * NCHUNK * TTILE], BF16)
    di("veb", [128, (T // 128) * 256], BF16)
    di("wqk", [128, NCHUNK * 512], BF16)
    di("wv", [128, NCHUNK * 260], BF16)
    di("wga36", [128, NCHUNK * 36], BF16)
    di("wo", [128, 2 * 1024], BF16)
    di("cdup", [128, T], BF16)
    di("s2dup", [128, T], BF16)
    di("qaug", [AUG, T], BF16)
    di("kaug", [AUG, T], BF16)
    di("onesEt", [128, NHEADS * 8], BF16)
    di("e8sel", [8, NHEADS * 128], BF16)
    di("e4", [2, 4, 128], BF16)
    di("negio", [128, 512], F32)
    di("negp", [128, 1], F32)
    dr["out"] = nc.dram_tensor("out", [T, D], F32, kind="ExternalOutput").ap()
    with tile.TileContext(nc) as tc:
        build_kernel(tc, dr, spans, alpha)
    nc.compile()
    _CACHE[key] = nc
    return nc


def kernel(x, ve, sa_lambdas, cos, sin, qkvo_w, attn_gate_w, ve_gate_w,
           attn_scale, docs):
    x = np.asarray(x, dtype=np.float32)
    ve = np.asarray(ve, dtype=np.float32)
    sa_lambdas = np.asarray(sa_lambdas, dtype=np.float32)
    cos = np.asarray(cos, dtype=np.float32)
    sin = np.asarray(sin, dtype=np.float32)
    qkvo_w = np.asarray(qkvo_w, dtype=np.float32)
    attn_gate_w = np.asarray(attn_gate_w, dtype=np.float32)
    ve_gate_w = np.asarray(ve_gate_w, dtype=np.float32)
    docs = np.asarray(docs, dtype=np.int32)
    alpha = float(np.asarray(attn_scale))

    segs = []
    s = 0
    for t in range(1, T + 1):
        if t == T or docs[t] != docs[t - 1]:
            segs.append((s, t))
            s = t
    spans = build_spans(segs)
    nc = _get_program((tuple(segs), alpha), spans, alpha)

    lam0, lam1 = float(sa_lambdas[0]), float(sa_lambdas[1])

    cosT = np.ascontiguousarray(cos.T)
    sinT = np.ascontiguousarray(sin.T)
    cblk = np.concatenate([cosT[0:16], cosT[0:16], cosT[16:32], cosT[16:32]],
                          axis=0)
    sblk = np.concatenate([-sinT[0:16], sinT[0:16], -sinT[16:32],
                           sinT[16:32]], axis=0)
    cdup = np.tile(cblk, (2, 1)).astype(BF)
    s2dup = np.tile(sblk, (2, 1)).astype(BF)
    onehot = (docs[None, :] == np.arange(NDOC)[:, None]).astype(np.float32)
    kaug = np.concatenate([onehot, np.ones((1, T), np.float32)],
                          axis=0).astype(BF)
    qaug = np.concatenate(
        [(BIG / alpha) * onehot, -(BIG / alpha) * np.ones((1, T), np.float32)],
        axis=0).astype(BF)
    onesEt = np.zeros((128, NHEADS, 8), np.float32)
    e8sel = np.zeros((8, NHEADS, 128), np.float32)
    for b in range(NHEADS):
        onesEt[0:64, b, 2 * b] = 1.0
        onesEt[64:128, b, 2 * b + 1] = 1.0
        e8sel[2 * b, b, 0:64] = 1.0
        e8sel[2 * b + 1, b, 64:128] = 1.0
    onesEt = onesEt.reshape(128, -1).astype(BF)
    e8sel = e8sel.reshape(8, -1).astype(BF)
    e4 = np.zeros((2, 4, 128), np.float32)
    e4[0, 0, 0:64] = 1.0
    e4[0, 1, 64:128] = 1.0
    e4[1, 2, 0:64] = 1.0
    e4[1, 3, 64:128] = 1.0
    e4 = e4.astype(BF)
    negio = np.broadcast_to(-np.arange(512, dtype=np.float32), (128, 512))
    negio = np.ascontiguousarray(negio)
    negp = -np.arange(128, dtype=np.float32).reshape(128, 1)
    negp = np.ascontiguousarray(negp)

    Wq, Wk, Wv, Wo = (qkvo_w[0:D], qkvo_w[D:2 * D], qkvo_w[2 * D:3 * D],
                      qkvo_w[3 * D:4 * D])

    in_maps = []
    for core in range(8):
        b = core // HGROUPS
        hg = core % HGROUPS
        heads = list(range(hg * NHEADS, (hg + 1) * NHEADS))
        perm = np.r_[0:16, 32:48, 16:32, 48:64]
        blocks = []
        for h in heads:
            blocks.append(lam0 * Wq[h * HD:(h + 1) * HD][perm].T)
            blocks.append(lam0 * Wk[h * HD:(h + 1) * HD][perm].T)
        wqk = np.concatenate(blocks, axis=1).astype(np.float32)
        wqk = np.ascontiguousarray(
            wqk.reshape(NCHUNK, 128, 512).transpose(1, 0, 2)
            .reshape(128, -1)).astype(BF)
        wv_cols = [lam0 * Wv[h * HD:(h + 1) * HD].T for h in heads]
        wv_cols.append(ve_gate_w[heads].T)
        wv = np.concatenate(wv_cols, axis=1).astype(np.float32)
        wv = np.ascontiguousarray(
            wv.reshape(NCHUNK, 128, 260).transpose(1, 0, 2)
            .reshape(128, -1)).astype(BF)
        wga36 = np.zeros((D, 36), np.float32)
        wga36[:, 32:36] = attn_gate_w[heads].T
        wga36 = np.ascontiguousarray(
            wga36.reshape(NCHUNK, 128, 36).transpose(1, 0, 2)
            .reshape(128, -1)).astype(BF)
        wo = (lam1 * Wo[:, hg * 256:(hg + 1) * 256].T).astype(np.float32)
        wo = np.ascontiguousarray(
            wo.reshape(2, 128, 1024).transpose(1, 0, 2)
            .reshape(128, -1)).astype(BF)
        xTn = x[b].T.astype(np.float32)  # [D, T]
        xT = np.ascontiguousarray(
            xTn.reshape(NCHUNK, 128, NTT, TTILE).transpose(1, 2, 0, 3)
            .reshape(128, -1)).astype(BF)
        veb = np.ascontiguousarray(
            (VE_GATE_SCALE * ve[b, :, hg * 256:(hg + 1) * 256])
            .reshape(T // 128, 128, 256).transpose(1, 0, 2)
            .reshape(128, -1)).astype(BF)
        in_maps.append({
            "xT": xT, "veb": veb, "wqk": wqk, "wv": wv, "wga36": wga36,
            "wo": wo, "cdup": cdup, "s2dup": s2dup, "qaug": qaug,
            "kaug": kaug, "onesEt": onesEt, "e8sel": e8sel, "e4": e4,
            "negio": negio, "negp": negp,
        })

    global LAST_RESULT
    res = run_bass_kernel_spmd(nc, in_maps, list(range(8)), trace=TRACE)
    LAST_RESULT = res
    out = np.zeros((B, T, D), dtype=np.float32)
    for core in range(8):
        out[core // HGROUPS] += res.results[core]["out"]
    return out


# revision 31
# speedup vs baseline: 1.0784x; 1.0784x over previous
"""Trainium2 Bass kernel for nn_CausalSelfAttention_59253368815644.

Sharding: 8 cores = 2 (batch) x 4 (head groups of 4 heads). Per core:
qkv projection (bf16 matmuls, FWL), rms-norm via DVE squares + PE
partition-sums + ACT ln/exp (single activation table set), rotary in bf16
on DVE with direct writes into persistent Q/K tiles (KEY_OFFSET via
split-destination writes -- no scatter DMAs), doc-masked causal attention
(one-hot augmented contraction rows; causal via affine_select), softmax
without max-subtraction, gated value embedding, attention output gate
(sigmoid via exp + fast reciprocal), partial output projection. Host sums
4 partials per batch element.
"""
import sys

sys.path.insert(0, "/opt/trn_rl_repo")

from contextlib import ExitStack

import ml_dtypes
import numpy as np

import concourse.bass as bass
import concourse.tile as tile
from concourse import bacc, mybir
from concourse._compat import with_exitstack
from concourse.bass_utils import run_bass_kernel_spmd

F32 = mybir.dt.float32
F32R = mybir.dt.float32r
BF16 = mybir.dt.bfloat16
AF = mybir.ActivationFunctionType
BF = ml_dtypes.bfloat16

B, T, D, H, HD = 2, 2048, 1024, 16, 64
EPS = 1.1920929e-07
VE_GATE_SCALE = 2.0
NHEADS = 4          # heads per core
HGROUPS = 4
NCHUNK = D // 128   # 8 contraction chunks
TTILE = 512
NTT = T // TTILE
BIG = 30.0          # mask exponent after exp-scale
NDOC = 8
AUG = NDOC + 1
QR = 64 + AUG       # 73 partitions for Q^/K^


def build_spans(segs):
    """Greedy partition of [0,T) into q-spans (len 256..512 where possible),
    preferring doc-boundary ends. Returns [(a, b, kts)]."""
    bounds = [e for (_, e) in segs]
    spans = []
    a = 0
    while a < T:
        cands = [e for e in bounds if a < e <= a + 512]
        end = None
        if cands:
            mx = max(cands)
            if mx - a >= 256 or mx == T:
                end = mx
        if end is None:
            end = min(a + 512, T)
        if end % 2 != 0 and end < T:
            end += 1
        ks = max((s for (s, _) in segs if s <= a), default=0)
        spans.append((a, end, ks))
        a = end
    def docend(pos):
        for (s_, e_) in segs:
            if s_ <= pos < e_:
                return e_
        return T

    out = []
    for (a, b, ks) in spans:
        ka0 = (ks // 128) * 128
        kts = []
        ka = ka0
        while ka < b:
            kn = min(128, b - ka)
            # q-columns beyond the last key row's document are fully masked
            qhi = min(b - a, docend(ka + kn - 1) - a)
            w0 = max(0, ka - a)
            if qhi > w0:
                kts.append((ka, kn, (ka + kn) > a, qhi))
            ka += 128
        out.append((a, b, kts))
    return out


@with_exitstack
def build_kernel(ctx: ExitStack, tc: tile.TileContext, dr, spans, alpha):
    nc = tc.nc

    const = ctx.enter_context(tc.tile_pool(name="const", bufs=1))
    persist = ctx.enter_context(tc.tile_pool(name="persist", bufs=1))

    # ---- constants / inputs staged in SBUF ----
    # queue order matters: wqk + x tile 0 first so the first qk matmuls can
    # start as early as possible; veb is not needed until the first vps gate
    wqk = const.tile([128, NCHUNK, 512], BF16)
    nc.scalar.dma_start(wqk[:],
                        dr["wqk"][:].rearrange("p (c e) -> p c e", e=512))
    wv = const.tile([128, NCHUNK, 260], BF16)
    nc.gpsimd.dma_start(wv[:],
                        dr["wv"][:].rearrange("p (c e) -> p c e", e=260))
    wga36 = const.tile([128, NCHUNK, 36], BF16)
    nc.gpsimd.dma_start(wga36[:],
                        dr["wga36"][:].rearrange("p (c e) -> p c e", e=36))
    xall = const.tile([128, NTT, NCHUNK, TTILE], BF16)
    xr = dr["xT"][:].rearrange("p (u c t) -> p u c t", c=NCHUNK, t=TTILE)
    for u in range(NTT):
        (nc.sync, nc.scalar)[u % 2].dma_start(xall[:, u, :, :], xr[:, u, :, :])
    vesb = const.tile([128, T // 128, 256], BF16)
    nc.scalar.dma_start(
        vesb[:], dr["veb"][:].rearrange("p (s e) -> p s e", e=256))
    cdup = const.tile([128, T], BF16)
    nc.gpsimd.dma_start(cdup[:], dr["cdup"][:])
    s2dup = const.tile([128, T], BF16)
    nc.gpsimd.dma_start(s2dup[:], dr["s2dup"][:])
    onesEt = const.tile([128, NHEADS, 8], BF16)
    nc.scalar.dma_start(onesEt[:],
                        dr["onesEt"][:].rearrange("p (b e) -> p b e", e=8))
    e8sel = const.tile([8, NHEADS, 128], BF16)
    nc.scalar.dma_start(e8sel[:],
                        dr["e8sel"][:].rearrange("p (b e) -> p b e", e=128))
    e4a = const.tile([4, 128], BF16)
    nc.scalar.dma_start(e4a[:], dr["e4"][0, :, :])
    e4b = const.tile([4, 128], BF16)
    nc.scalar.dma_start(e4b[:], dr["e4"][1, :, :])
    epsb = const.tile([8, 1], F32)
    nc.vector.memset(epsb[:], EPS)
    negio = const.tile([128, 512], F32)
    nc.sync.dma_start(negio[:], dr["negio"][:])
    negp = const.tile([128, 1], F32)
    nc.sync.dma_start(negp[:], dr["negp"][:])
    wo = const.tile([128, 2, 1024], BF16)
    nc.sync.dma_start(wo[:],
                      dr["wo"][:].rearrange("p (c e) -> p c e", e=1024))

    # ---- persistent activations ----
    Qh = persist.tile([QR, NHEADS, T], BF16)
    Kh = persist.tile([QR, NHEADS, T], BF16)
    qaug = dr["qaug"]
    kaug = dr["kaug"]
    nc.gpsimd.dma_start(
        Qh[64:QR, :, :],
        bass.AP(tensor=qaug.tensor, offset=qaug.offset,
                ap=[[T, AUG], [0, NHEADS], [1, T]]))
    nc.gpsimd.dma_start(
        Kh[64:QR, :, :],
        bass.AP(tensor=kaug.tensor, offset=kaug.offset,
                ap=[[T, AUG], [0, NHEADS], [1, T]]))
    Vh = persist.tile([128, T // 128, NHEADS, 65], BF16)
    nc.vector.memset(Vh[:, :, :, 64:65], 1.0)
    agrow1 = persist.tile([NHEADS, T], F32)   # 1 + exp(-attn_gate_logit)

    # =========== Phase 1 ===========
    with tc.tile_pool(name="p1qk", bufs=1, space="PSUM") as qkps_pool, \
         tc.tile_pool(name="p1ss", bufs=1, space="PSUM") as ssps_pool, \
         tc.tile_pool(name="p1v", bufs=2, space="PSUM") as vps_pool, \
         tc.tile_pool(name="p1rb", bufs=1, space="PSUM") as rbps_pool, \
         tc.tile_pool(name="p1sb", bufs=2) as sb_pool, \
         tc.tile_pool(name="p1n", bufs=2) as n1_pool:
        for tt in range(NTT):
            t0 = tt * TTILE

            # qkv projection for 4 heads (q|k interleaved per head)
            qk = qkps_pool.tile([128, NHEADS, TTILE], F32, tag="qk")
            for blk in range(NHEADS):
                for c in range(NCHUNK):
                    nc.tensor.matmul(
                        qk[:, blk, :], wqk[:, c, blk * 128:(blk + 1) * 128],
                        xall[:, tt, c, :],
                        start=(c == 0), stop=(c == NCHUNK - 1))
            qkb = sb_pool.tile([128, NHEADS, TTILE], BF16, tag="qkb")
            nc.scalar.activation(out=qkb[:], in_=qk[:], func=AF.Copy,
                                 scale=1.0)
            sq = sb_pool.tile([128, NHEADS, TTILE], BF16, tag="sq")
            nc.vector.tensor_mul(sq[:], qkb[:], qkb[:])

            # attn-gate logits (rows 32:36) + per-head sum-squares (rows 0:8)
            ss8z = ssps_pool.tile([36, TTILE], F32, tag="ss")
            for c in range(NCHUNK):
                nc.tensor.matmul(ss8z[:], wga36[:, c, :], xall[:, tt, c, :],
                                 start=(c == 0), stop=False)
            for blk in range(NHEADS):
                nc.tensor.matmul(ss8z[0:8, :], onesEt[:, blk, :],
                                 sq[:, blk, :],
                                 start=False, stop=(blk == NHEADS - 1))
            lnss = n1_pool.tile([8, TTILE], F32, tag="lnss")
            nc.scalar.activation(out=lnss[:], in_=ss8z[0:8, :], func=AF.Ln,
                                 scale=1.0 / HD, bias=epsb[:])
            rstd8 = n1_pool.tile([8, TTILE], BF16, tag="rstd8")
            nc.scalar.activation(out=rstd8[:], in_=lnss[:], func=AF.Exp,
                                 scale=-0.5)
            age = n1_pool.tile([NHEADS, TTILE], F32, tag="age")
            nc.scalar.activation(out=age[:], in_=ss8z[32:36, :], func=AF.Exp,
                                 scale=-1.0)
            nc.vector.tensor_scalar_add(agrow1[:, t0:t0 + TTILE], age[:], 1.0)

            for blk in range(NHEADS):
                rbps = rbps_pool.tile([128, TTILE], F32, tag="rb")
                nc.tensor.matmul(rbps[:], e8sel[:, blk, :], rstd8[:],
                                 start=True, stop=True)
                rstdb = n1_pool.tile([128, TTILE], BF16, tag="rstdb")
                nc.scalar.activation(out=rstdb[:], in_=rbps[:], func=AF.Copy,
                                     scale=1.0)
                A = sb_pool.tile([128, TTILE], BF16, tag="A")
                nc.vector.tensor_mul(A[:], qkb[:, blk, :],
                                     cdup[:, t0:t0 + TTILE])
                Bt = sb_pool.tile([128, TTILE], BF16, tag="B")
                nc.vector.tensor_mul(Bt[:], qkb[:, blk, :],
                                     s2dup[:, t0:t0 + TTILE])
                Bs = sb_pool.tile([128, TTILE], BF16, tag="Bs")
                nc.vector.stream_shuffle(Bs[:], Bt[:],
                                         mask=[g ^ 16 for g in range(32)])
                rotr = sb_pool.tile([128, TTILE], BF16, tag="rotr")
                nc.vector.tensor_add(rotr[:], A[:], Bs[:])
                h = blk
                nc.vector.tensor_mul(Qh[0:64, h, t0:t0 + TTILE],
                                     rotr[0:64, :], rstdb[0:64, :])
                nc.vector.tensor_mul(Kh[0:32, h, t0:t0 + TTILE],
                                     rotr[64:96, :], rstdb[64:96, :])
                w = TTILE if t0 + TTILE < T else TTILE - 1
                nc.vector.tensor_mul(Kh[32:64, h, t0 + 1:t0 + 1 + w],
                                     rotr[96:128, 0:w], rstdb[96:128, 0:w])
                if t0 == 0:
                    nc.vector.tensor_mul(Kh[32:64, h, 0:1],
                                         rotr[96:128, 0:1], rstdb[96:128, 0:1])

            # value projection + gated ve
            for sub in range(TTILE // 128):
                stg = (t0 + sub * 128) // 128
                vps = vps_pool.tile([128, 260], F32, tag="v")
                for c in range(NCHUNK):
                    nc.tensor.matmul(
                        vps[:], xall[:, tt, c, sub * 128:(sub + 1) * 128],
                        wv[:, c, :],
                        start=(c == 0), stop=(c == NCHUNK - 1))
                ge = n1_pool.tile([128, NHEADS], F32, tag="ge")
                nc.scalar.activation(out=ge[:], in_=vps[:, 256:260],
                                     func=AF.Exp, scale=-1.0)
                nc.vector.tensor_scalar_add(ge[:], ge[:], 1.0)
                gf = n1_pool.tile([128, NHEADS], F32, tag="gf")
                nc.vector.reciprocal_approx_fast(out=gf[:], in_=ge[:])
                gb16 = n1_pool.tile([128, NHEADS], BF16, tag="gb16")
                nc.vector.tensor_copy(gb16[:], gf[:])
                gap = gb16[:]
                gb = bass.AP(tensor=gap.tensor, offset=gap.offset,
                             ap=[list(gap.ap[0]), [1, NHEADS], [0, HD]])
                tmp = n1_pool.tile([128, NHEADS, HD], BF16, tag="vtmp")
                nc.gpsimd.tensor_mul(
                    tmp[:],
                    vesb[:, stg, :].rearrange("p (h d) -> p h d", h=NHEADS),
                    gb)
                nc.vector.tensor_add(
                    Vh[:, stg, :, 0:64],
                    vps[:, 0:256].rearrange("p (h d) -> p h d", h=NHEADS),
                    tmp[:])

    # =========== Phase 2 (attention + interleaved o-proj) ===========
    ypool = ctx.enter_context(tc.tile_pool(name="ylate", bufs=1))
    y01 = ypool.tile([128, T], BF16)
    y23 = ypool.tile([128, T], BF16)

    def oproj(ti, ops_pool, osb_pool):
        tt0 = ti * 128
        for eh in range(2):
            ops = ops_pool.tile([128, 512], F32, tag="o")
            nc.tensor.matmul(ops[:], y01[:, tt0:tt0 + 128],
                             wo[:, 0, eh * 512:(eh + 1) * 512],
                             start=True, stop=False)
            nc.tensor.matmul(ops[:], y23[:, tt0:tt0 + 128],
                             wo[:, 1, eh * 512:(eh + 1) * 512],
                             start=False, stop=True)
            osb = osb_pool.tile([128, 512], F32, tag="osb")
            nc.vector.tensor_copy(osb[:], ops[:])
            nc.sync.dma_start(
                dr["out"][tt0:tt0 + 128, eh * 512:(eh + 1) * 512], osb[:])

    with tc.tile_pool(name="p2s", bufs=2, space="PSUM") as sps_pool, \
         tc.tile_pool(name="p2y", bufs=1, space="PSUM") as yps_pool, \
         tc.tile_pool(name="p3ps", bufs=2, space="PSUM") as ops_pool, \
         tc.tile_pool(name="p3sb", bufs=3) as osb_pool, \
         tc.tile_pool(name="p2p", bufs=6) as pt_pool, \
         tc.tile_pool(name="p2sc", bufs=2) as sc_pool:
        bps_pool = ops_pool
        pending = []
        onext = 0
        for (a, b_, kts) in spans:
            N = b_ - a
            ypss = []
            for h in range(NHEADS):
                # paced o-proj filler: one finished 128-token tile per head
                # chain keeps the PE fed through exp/affine stalls
                if pending:
                    oproj(pending.pop(0), ops_pool, osb_pool)
                yps = yps_pool.tile([65, 512], F32, tag=f"y{h}")
                ypss.append(yps)
                for ki, (ka, kn, causal, qhi) in enumerate(kts):
                    w0 = max(0, ka - a)
                    sps = sps_pool.tile([128, 512], F32, tag="s")
                    nc.tensor.matmul(sps[0:kn, w0:qhi],
                                     Kh[:, h, ka:ka + kn],
                                     Qh[:, h, a + w0:a + qhi],
                                     start=True, stop=True)
                    pt = pt_pool.tile([128, 512], BF16, tag="p")
                    nc.scalar.activation(out=pt[0:kn, w0:qhi],
                                         in_=sps[0:kn, w0:qhi],
                                         func=AF.Exp, scale=alpha)
                    if causal:
                        bw = min(qhi, ka + kn - a) - w0
                        if bw > 0:
                            nc.gpsimd.affine_select(
                                out=pt[0:kn, w0:w0 + bw],
                                in_=pt[0:kn, w0:w0 + bw],
                                compare_op=mybir.AluOpType.is_ge,
                                fill=0.0, base=a + w0 - ka,
                                pattern=[[1, bw]], channel_multiplier=-1)
                    nc.tensor.matmul(yps[:, w0:qhi],
                                     Vh[0:kn, ka // 128, h, :],
                                     pt[0:kn, w0:qhi],
                                     start=(ki == 0), stop=(ki == len(kts) - 1))
            l4 = sc_pool.tile([NHEADS, 512], F32, tag="l4")
            for h in range(NHEADS):
                l1 = sc_pool.tile([1, 512], F32, tag=f"l1_{h}")
                nc.vector.tensor_copy(l1[:, 0:N], ypss[h][64:65, 0:N])
                nc.sync.dma_start(l4[h:h + 1, 0:N], l1[:, 0:N])
            mm = sc_pool.tile([NHEADS, 512], F32, tag="mm")
            nc.vector.tensor_mul(mm[:, 0:N], l4[:, 0:N], agrow1[:, a:b_])
            scf = sc_pool.tile([NHEADS, 512], F32, tag="scf")
            nc.vector.reciprocal_approx_fast(out=scf[:, 0:N], in_=mm[:, 0:N])
            sc4 = sc_pool.tile([NHEADS, 512], BF16, tag="sc")
            nc.vector.tensor_copy(sc4[:, 0:N], scf[:, 0:N])
            for pr, ytile in ((0, y01), (1, y23)):
                sbcps = bps_pool.tile([128, 512], F32, tag="o")
                nc.tensor.matmul(sbcps[:, 0:N], e4a[:] if pr == 0 else e4b[:],
                                 sc4[:, 0:N], start=True, stop=True)
                sbcs = sc_pool.tile([128, 512], BF16, tag="sbcs")
                nc.scalar.activation(out=sbcs[:, 0:N], in_=sbcps[:, 0:N],
                                     func=AF.Copy, scale=1.0)
                yy = sc_pool.tile([128, 512], BF16, tag="yy")
                nc.vector.tensor_copy(yy[0:64, 0:N],
                                      ypss[2 * pr][0:64, 0:N])
                nc.vector.tensor_copy(yy[64:128, 0:N],
                                      ypss[2 * pr + 1][0:64, 0:N])
                nc.vector.tensor_mul(ytile[:, a:b_], yy[:, 0:N],
                                     sbcs[:, 0:N])
            pending.extend(range(onext, b_ // 128))
            onext = b_ // 128
        for ti in pending:
            oproj(ti, ops_pool, osb_pool)


_CACHE = {}
TRACE = False       # set by test harness to capture an NTFF profile
LAST_RESULT = None  # BassKernelResults of the most recent run


def _get_program(key, spans, alpha):
    if key in _CACHE:
        return _CACHE[key]
    nc = bacc.Bacc("TRN2", target_bir_lowering=False, debug=False)
    dr = {}

    def di(name, shape, dt=F32):
        dr[name] = nc.dram_tensor(name, shape, dt, kind="ExternalInput").ap()

    di("xT", [128, NTT * NCHUNK * TTILE], BF16)
    di("veb", [128, (T // 128) * 256], BF16)
    di("wqk", [128, NCHUNK * 512], BF16)
    di("wv", [128, NCHUNK * 260], BF16)
    di("wga36", [128, NCHUNK * 36], BF16)
    di("wo", [128, 2 * 1024], BF16)
    di("cdup", [128, T], BF16)
    di("s2dup", [128, T], BF16)
    di("qaug", [AUG, T], BF16)
    di("kaug", [AUG, T], BF16)
    di("onesEt", [128, NHEADS * 8], BF16)
    di("e8sel", [8, NHEADS * 128], BF16)
    di("e4", [2, 4, 128], BF16)
    di("negio", [128, 512], F32)
    di("negp", [128, 1], F32)
    dr["out"] = nc.dram_tensor("out", [T, D], F32, kind="ExternalOutput").ap()
    with tile.TileContext(nc) as tc:
        build_kernel(tc, dr, spans, alpha)
    nc.compile()
    _CACHE[key] = nc
    return nc


def kernel(x, ve, sa_lambdas, cos, sin, qkvo_w, attn_gate_w, ve_gate_w,
           attn_scale, docs):
    x = np.asarray(x, dtype=np.float32)
    ve = np.asarray(ve, dtype=np.float32)
    sa_lambdas = np.asarray(sa_lambdas, dtype=np.float32)
    cos = np.asarray(cos, dtype=np.float32)
    sin = np.asarray(sin, dtype=np.float32)
    qkvo_w = np.asarray(qkvo_w, dtype=np.float32)
    attn_gate_w = np.asarray(attn_gate_w, dtype=np.float32)
    ve_gate_w = np.asarray(ve_gate_w, dtype=np.float32)
    docs = np.asarray(docs, dtype=np.int32)
    alpha = float(np.asarray(attn_scale))

    segs = []
    s = 0
    for t in range(1, T + 1):
        if t == T or docs[t] != docs[t - 1]:
            segs.append((s, t))
            s = t
    spans = build_spans(segs)
    nc = _get_program((tuple(segs), alpha), spans, alpha)

    lam0, lam1 = float(sa_lambdas[0]), float(sa_lambdas[1])

    cosT = np.ascontiguousarray(cos.T)
    sinT = np.ascontiguousarray(sin.T)
    cblk = np.concatenate([cosT[0:16], cosT[0:16], cosT[16:32], cosT[16:32]],
                          axis=0)
    sblk = np.concatenate([-sinT[0:16], sinT[0:16], -sinT[16:32],
                           sinT[16:32]], axis=0)
    cdup = np.tile(cblk, (2, 1)).astype(BF)
    s2dup = np.tile(sblk, (2, 1)).astype(BF)
    onehot = (docs[None, :] == np.arange(NDOC)[:, None]).astype(np.float32)
    kaug = np.concatenate([onehot, np.ones((1, T), np.float32)],
                          axis=0).astype(BF)
    qaug = np.concatenate(
        [(BIG / alpha) * onehot, -(BIG / alpha) * np.ones((1, T), np.float32)],
        axis=0).astype(BF)
    onesEt = np.zeros((128, NHEADS, 8), np.float32)
    e8sel = np.zeros((8, NHEADS, 128), np.float32)
    for b in range(NHEADS):
        onesEt[0:64, b, 2 * b] = 1.0
        onesEt[64:128, b, 2 * b + 1] = 1.0
        e8sel[2 * b, b, 0:64] = 1.0
        e8sel[2 * b + 1, b, 64:128] = 1.0
    onesEt = onesEt.reshape(128, -1).astype(BF)
    e8sel = e8sel.reshape(8, -1).astype(BF)
    e4 = np.zeros((2, 4, 128), np.float32)
    e4[0, 0, 0:64] = 1.0
    e4[0, 1, 64:128] = 1.0
    e4[1, 2, 0:64] = 1.0
    e4[1, 3, 64:128] = 1.0
    e4 = e4.astype(BF)
    negio = np.broadcast_to(-np.arange(512, dtype=np.float32), (128, 512))
    negio = np.ascontiguousarray(negio)
    negp = -np.arange(128, dtype=np.float32).reshape(128, 1)
    negp = np.ascontiguousarray(negp)

    Wq, Wk, Wv, Wo = (qkvo_w[0:D], qkvo_w[D:2 * D], qkvo_w[2 * D:3 * D],
                      qkvo_w[3 * D:4 * D])

    in_maps = []
    for core in range(8):
        b = core // HGROUPS
        hg = core % HGROUPS
        heads = list(range(hg * NHEADS, (hg + 1) * NHEADS))
        perm = np.r_[0:16, 32:48, 16:32, 48:64]
        blocks = []
        for h in heads:
            blocks.append(lam0 * Wq[h * HD:(h + 1) * HD][perm].T)
            blocks.append(lam0 * Wk[h * HD:(h + 1) * HD][perm].T)
        wqk = np.concatenate(blocks, axis=1).astype(np.float32)
        wqk = np.ascontiguousarray(
            wqk.reshape(NCHUNK, 128, 512).transpose(1, 0, 2)
            .reshape(128, -1)).astype(BF)
        wv_cols = [lam0 * Wv[h * HD:(h + 1) * HD].T for h in heads]
        wv_cols.append(ve_gate_w[heads].T)
        wv = np.concatenate(wv_cols, axis=1).astype(np.float32)
        wv = np.ascontiguousarray(
            wv.reshape(NCHUNK, 128, 260).transpose(1, 0, 2)
            .reshape(128, -1)).astype(BF)
        wga36 = np.zeros((D, 36), np.float32)
        wga36[:, 32:36] = attn_gate_w[heads].T
        wga36 = np.ascontiguousarray(
            wga36.reshape(NCHUNK, 128, 36).transpose(1, 0, 2)
            .reshape(128, -1)).astype(BF)
        wo = (lam1 * Wo[:, hg * 256:(hg + 1) * 256].T).astype(np.float32)
        wo = np.ascontiguousarray(
            wo.reshape(2, 128, 1024).transpose(1, 0, 2)
            .reshape(128, -1)).astype(BF)
        xTn = x[b].T.astype(np.float32)  # [D, T]
        xT = np.ascontiguousarray(
            xTn.reshape(NCHUNK, 128, NTT, TTILE).transpose(1, 2, 0, 3)
            .reshape(128, -1)).astype(BF)
        veb = np.ascontiguousarray(
            (VE_GATE_SCALE * ve[b, :, hg * 256:(hg + 1) * 256])
            .reshape(T // 128, 128, 256).transpose(1, 0, 2)
            .reshape(128, -1)).astype(BF)
        in_maps.append({
            "xT": xT, "veb": veb, "wqk": wqk, "wv": wv, "wga36": wga36,
            "wo": wo, "cdup": cdup, "s2dup": s2dup, "qaug": qaug,
            "kaug": kaug, "onesEt": onesEt, "e8sel": e8sel, "e4": e4,
            "negio": negio, "negp": negp,
        })

    global LAST_RESULT
    res = run_bass_kernel_spmd(nc, in_maps, list(range(8)), trace=TRACE)
    LAST_RESULT = res
    out = np.zeros((B, T, D), dtype=np.float32)
    for core in range(8):
        out[core // HGROUPS] += res.results[core]["out"]
    return out
